# revision 1
# baseline (speedup 1.0000x reference)
"""Multi-head self-attention (B=4, S=2048, D=768, H=12, dh=64) on 8 trn2 cores.

Sharding: core = b*2 + g  (b = batch 0..3, g = head-group of 6 heads).
Each core computes q/k/v projections for its 6 heads over the full sequence,
masked softmax attention, and a partial output projection (column slice of
o_w => row-parallel). Host sums the two partial outputs per batch element.

Key points:
  - mask gather: only unmasked k positions (padded to a multiple of 128) are
    shipped/projected/exp'd; padding columns get a -1e30 per-partition bias
    inside the ACT exp instruction (out = exp(scale*s + bias)).
  - scoresT [k, q] layout so softmax weights feed the context matmul as lhsT
    with no transpose; softmax denominators come free from an appended
    ones-column in v (psum row 64 of the context matmul).
  - ALL matmuls use a full K=128 contraction: the PE HAM clock gate only
    un-throttles (1.2 -> 2.4 GHz) for high row-utilization streams, so the
    64-dim per-head score contractions are zero-padded to 128 rows (kTz holds
    each head's kT in its own 64-row half, other half zero), and the rank-1
    recip/bias broadcasts use a [128, M] ones-row matrix against an input
    whose rows 1..127 are zeroed.
  - per-head-per-qhalf normalization: sums row evicted, approx reciprocal
    (~18-bit, 5x faster than exact), broadcast into rows 64..127 of the ctx
    psum tile via the K=128 ones-row matmul, one tensor_tensor multiply.
  - biases: q/k bias = per-partition DVE tensor_scalar on psum eviction;
    v bias via contraction-augmentation (ones row in xvT, v_b row in wvT);
    o_b broadcast across partitions once, added on psum evict (zeros passed
    for the g==1 cores so the host sum applies it once).
"""

import numpy as np
import ml_dtypes

import concourse.bass as bass
import concourse.mybir as mybir
import concourse.tile as tile
from concourse import bacc
from concourse.bass_utils import run_bass_kernel_spmd

BS, SEQ, DIM, NH = 4, 2048, 768, 12
DH = 64
HEADS = 6            # heads per core
DGRP = HEADS * DH    # 384
N_CORES = 8
P = 128
QH = 1024            # q-half width in the attention loop

F32 = mybir.dt.float32
BF16 = mybir.dt.bfloat16

MM_DT = BF16
MM_NP = ml_dtypes.bfloat16 if MM_DT == BF16 else np.float32

NEG = -1.0e30


def _build(NKV: int):
    """Build the per-core Bass program, parameterized by padded kv length."""
    KC = NKV // P          # k chunks
    QC = SEQ // P          # 16
    NT = SEQ // 512        # 4
    KIN = DIM // P         # 6 contraction chunks for q/k proj
    KIN_V = 7              # 768 inputs + ones row, padded to 896

    nc = bacc.Bacc(None, target_bir_lowering=False, debug=False)

    xqT = nc.declare_dram_parameter("xqT", [DIM, SEQ], MM_DT, isOutput=False)
    xkT = nc.declare_dram_parameter("xkT", [DIM, NKV], MM_DT, isOutput=False)
    xvT = nc.declare_dram_parameter("xvT", [P * KIN_V, NKV], MM_DT, isOutput=False)
    wqT = nc.declare_dram_parameter("wqT", [DIM, DGRP], MM_DT, isOutput=False)
    wkT = nc.declare_dram_parameter("wkT", [DIM, DGRP], MM_DT, isOutput=False)
    wvT = nc.declare_dram_parameter("wvT", [P * KIN_V, DGRP], MM_DT, isOutput=False)
    woT = nc.declare_dram_parameter("woT", [DGRP, DIM], MM_DT, isOutput=False)
    qb = nc.declare_dram_parameter("qb", [DGRP], F32, isOutput=False)
    kb = nc.declare_dram_parameter("kb", [DGRP], F32, isOutput=False)
    ob = nc.declare_dram_parameter("ob", [DIM], F32, isOutput=False)
    pb = nc.declare_dram_parameter("pb", [NKV], F32, isOutput=False)
    out = nc.declare_dram_parameter("out", [SEQ, DIM], F32, isOutput=True)

    xqT_r = xqT.rearrange("(kk pi) n -> pi kk n", pi=P)
    xkT_r = xkT.rearrange("(kk pi) n -> pi kk n", pi=P)
    xvT_r = xvT.rearrange("(kk pi) n -> pi kk n", pi=P)
    wqT_r = wqT.rearrange("(kk pi) n -> pi kk n", pi=P)
    wkT_r = wkT.rearrange("(kk pi) n -> pi kk n", pi=P)
    wvT_r = wvT.rearrange("(kk pi) n -> pi kk n", pi=P)
    woT_r = woT.rearrange("(kk pi) n -> pi kk n", pi=P)
    qb_r = qb.rearrange("(m pi) -> pi m", pi=P)
    kb_r = kb.rearrange("(m pi) -> pi m", pi=P)
    pb_r = pb.rearrange("(c pi) -> pi c", pi=P)

    with tile.TileContext(nc) as tc:
        with (
            tc.tile_pool(name="const", bufs=1) as const,
            tc.tile_pool(name="persist", bufs=1) as persist,
            tc.tile_pool(name="expp", bufs=3) as expp,
            tc.tile_pool(name="outp", bufs=3) as outp,
        ):
            # ---- constants ----
            pb_sb = const.tile([P, KC], F32)
            nc.sync.dma_start(pb_sb[:], pb_r)
            qb_sb = const.tile([P, 3], F32)
            nc.sync.dma_start(qb_sb[:], qb_r)
            kb_sb = const.tile([P, 3], F32)
            nc.sync.dma_start(kb_sb[:], kb_r)
            wo_sb = const.tile([P, 3, DIM], MM_DT)
            nc.sync.dma_start(wo_sb[:], woT_r)
            # ones-row matrix: row 0 all-ones, rows 1..127 zero. As lhsT this
            # replicates row 0 of the rhs into all M output partitions with a
            # full K=128 contraction (keeps the PE HAM clock warm).
            ones2_sb = const.tile([P, P], F32)
            nc.vector.memset(ones2_sb[:], 0.0)
            nc.vector.memset(ones2_sb[0:1, :], 1.0)
            ob_row = const.tile([P, DIM], F32)
            nc.vector.memset(ob_row[:], 0.0)
            nc.sync.dma_start(ob_row[0:1, :], ob[None, :])
            ob_bc = const.tile([P, DIM], F32)

            # ---- persistent activations ----
            qT_sb = persist.tile([P, 3, SEQ], MM_DT)
            # kTz: per head h, half 64*(h%2) holds kT_h, other half zero
            kTz_sb = persist.tile([P, HEADS, NKV], MM_DT)
            v_sb = persist.tile([P, KC, HEADS * 65], MM_DT)
            ctx_sb = persist.tile([P, 3, SEQ], MM_DT)
            # recip rows-zeroed tiles (row 0 = 1/sums, rows 1..127 = 0)
            recipA = persist.tile([P, QH], F32)
            recipB = persist.tile([P, QH], F32)

            nc.vector.memset(kTz_sb[:], 0.0)
            nc.vector.memset(recipA[:], 0.0)
            nc.vector.memset(recipB[:], 0.0)
            # ones column per head in v (gives softmax sums in psum row 64)
            for h in range(HEADS):
                nc.vector.memset(v_sb[:, :, 65 * h + 64], 1.0)

            with tc.tile_pool(name="wpool", bufs=1) as wpool, \
                 tc.tile_pool(name="xslice", bufs=3) as xslice, \
                 tc.tile_pool(name="psA", bufs=2, space="PSUM") as psA:
                # o_b broadcast across partitions
                for n0, nsz in ((0, 512), (512, 256)):
                    ps = psA.tile([P, 512], F32, tag="psA0")
                    nc.tensor.matmul(ps[:, 0:nsz], ones2_sb[:],
                                     ob_row[:, n0:n0 + nsz],
                                     start=True, stop=True)
                    nc.vector.tensor_copy(out=ob_bc[:, n0:n0 + nsz],
                                          in_=ps[:, 0:nsz])

                wq_sb = wpool.tile([P, KIN, DGRP], MM_DT)
                nc.sync.dma_start(wq_sb[:], wqT_r)
                wk_sb = wpool.tile([P, KIN, DGRP], MM_DT)
                nc.sync.dma_start(wk_sb[:], wkT_r)
                wv_sb = wpool.tile([P, KIN_V, DGRP], MM_DT)
                nc.sync.dma_start(wv_sb[:], wvT_r)

                # ---- q projection: qT[384, 2048] = wqT.T @ xqT (+qb) ----
                for nt in range(NT):
                    xq_t = xslice.tile([P, KIN, 512], MM_DT, tag="xq")
                    nc.sync.dma_start(xq_t[:], xqT_r[:, :, nt * 512:(nt + 1) * 512])
                    for m in range(3):
                        ps = psA.tile([P, 512], F32, tag=f"psA{(nt * 3 + m) % 2}")
                        for kk in range(KIN):
                            nc.tensor.matmul(
                                ps[:],
                                wq_sb[:, kk, m * P:(m + 1) * P],
                                xq_t[:, kk, :],
                                start=(kk == 0), stop=(kk == KIN - 1),
                            )
                        nc.vector.tensor_scalar_add(
                            qT_sb[:, m, nt * 512:(nt + 1) * 512], ps[:],
                            qb_sb[:, m, None],
                        )

                # ---- k projection into kTz (per-head 64-row halves) ----
                ksl = []
                o = 0
                while o < NKV:
                    w = min(512, NKV - o)
                    ksl.append((o, w))
                    o += w
                for nt, (o0, w) in enumerate(ksl):
                    xk_t = xslice.tile([P, KIN, 512], MM_DT, tag="xk")
                    nc.sync.dma_start(xk_t[:, :, 0:w], xkT_r[:, :, o0:o0 + w])
                    for m in range(3):
                        ps = psA.tile([P, 512], F32, tag=f"psA{(nt * 3 + m) % 2}")
                        for kk in range(KIN):
                            nc.tensor.matmul(
                                ps[:, 0:w],
                                wk_sb[:, kk, m * P:(m + 1) * P],
                                xk_t[:, kk, 0:w],
                                start=(kk == 0), stop=(kk == KIN - 1),
                            )
                        nc.vector.tensor_scalar_add(
                            kTz_sb[0:64, 2 * m, o0:o0 + w], ps[0:64, 0:w],
                            kb_sb[0:64, m, None],
                        )
                        nc.vector.tensor_scalar_add(
                            kTz_sb[64:128, 2 * m + 1, o0:o0 + w], ps[64:128, 0:w],
                            kb_sb[64:128, m, None],
                        )

                # ---- v projection: v[NKV, 384] (k rows on partitions) ----
                for m in range(KC):
                    xv_t = xslice.tile([P, KIN_V, P], MM_DT, tag="xv")
                    nc.sync.dma_start(xv_t[:], xvT_r[:, :, m * P:(m + 1) * P])
                    ps = psA.tile([P, DGRP], F32, tag=f"psA{m % 2}")
                    for kk in range(KIN_V):
                        nc.tensor.matmul(
                            ps[:],
                            xv_t[:, kk, :],
                            wv_sb[:, kk, :],
                            start=(kk == 0), stop=(kk == KIN_V - 1),
                        )
                    for h in range(HEADS):
                        nc.vector.tensor_copy(
                            out=v_sb[:, m, 65 * h:65 * h + 64],
                            in_=ps[:, 64 * h:64 * h + 64],
                        )

            # ---- attention, unit = (head, q-half) ----
            with tc.tile_pool(name="psS", bufs=2, space="PSUM") as psS, \
                 tc.tile_pool(name="psC", bufs=2, space="PSUM") as psC, \
                 tc.tile_pool(name="stat", bufs=2) as stat:
                def unit_tail(h, qh, ps_ctx, recip_t):
                    # normalization tail: sums -> approx recip -> K=128
                    # broadcast into rows 64..127 of the ctx psum tile -> one
                    # multiply. Emitted two kc-iterations into the NEXT unit
                    # so its DVE latency never blocks the PE/ACT stream.
                    chunk, off = h // 2, 64 * (h % 2)
                    q0 = qh * QH
                    sums_t = stat.tile([1, QH], F32, tag="sums")
                    nc.vector.tensor_copy(out=sums_t[:], in_=ps_ctx[64:65, :])
                    nc.vector.reciprocal_approx_fast(
                        out=recip_t[0:1, :], in_=sums_t[:])
                    ctxu_t = stat.tile([DH, QH], MM_DT, tag="ctxu")
                    nc.vector.tensor_copy(out=ctxu_t[:], in_=ps_ctx[0:64, :])
                    for qt in range(QH // 512):
                        nc.tensor.matmul(
                            ps_ctx[64:128, qt * 512:(qt + 1) * 512],
                            ones2_sb[:, 0:DH],
                            recip_t[:, qt * 512:(qt + 1) * 512],
                            start=True, stop=True,
                        )
                    nc.vector.tensor_tensor(
                        ctx_sb[off:off + DH, chunk, q0:q0 + QH],
                        ctxu_t[:],
                        ps_ctx[64:128, :],
                        mybir.AluOpType.mult,
                    )

                pending = None
                for u, (h, qh) in enumerate(
                        [(h, qh) for h in range(HEADS) for qh in range(SEQ // QH)]):
                    chunk, off = h // 2, 64 * (h % 2)
                    q0 = qh * QH
                    recip_t = recipA if u % 2 == 0 else recipB
                    ps_ctx = psC.tile([P, QH], F32, tag="ctx")
                    for kc in range(KC):
                        ps_s = psS.tile([P, QH], F32, tag="s")
                        for qt in range(QH // 512):
                            nc.tensor.matmul(
                                ps_s[:, qt * 512:(qt + 1) * 512],
                                kTz_sb[:, h, kc * P:(kc + 1) * P],
                                qT_sb[:, chunk,
                                      q0 + qt * 512:q0 + (qt + 1) * 512],
                                start=True, stop=True,
                            )
                        exp_t = expp.tile([P, QH], MM_DT, tag="exp")
                        nc.scalar.activation(
                            exp_t[:], ps_s[:], mybir.ActivationFunctionType.Exp,
                            bias=pb_sb[:, kc, None], scale=0.125,
                        )
                        for qt in range(QH // 512):
                            nc.tensor.matmul(
                                ps_ctx[0:65, qt * 512:(qt + 1) * 512],
                                v_sb[:, kc, 65 * h:65 * h + 65],
                                exp_t[:, qt * 512:(qt + 1) * 512],
                                start=(kc == 0), stop=(kc == KC - 1),
                            )
                        if kc == 1 and pending is not None:
                            pending()
                            pending = None
                    pending = (lambda h=h, qh=qh, ps_ctx=ps_ctx, recip_t=recip_t:
                               unit_tail(h, qh, ps_ctx, recip_t))
                pending()

            # ---- output projection: out[2048, 768] (+ o_b) ----
            with tc.tile_pool(name="psO", bufs=2, space="PSUM") as psO:
                for qc in range(QC):
                    o_t = outp.tile([P, DIM], F32, tag="o")
                    ps = psO.tile([P, DIM], F32, tag=f"psO{qc % 2}")
                    for kk in range(3):
                        for n0, nsz in ((0, 512), (512, 256)):
                            nc.tensor.matmul(
                                ps[:, n0:n0 + nsz],
                                ctx_sb[:, kk, qc * P:(qc + 1) * P],
                                wo_sb[:, kk, n0:n0 + nsz],
                                start=(kk == 0), stop=(kk == 2),
                            )
                    nc.vector.tensor_tensor(
                        o_t[:], ps[:], ob_bc[:], mybir.AluOpType.add,
                    )
                    nc.sync.dma_start(out[qc * P:(qc + 1) * P, :], o_t[:])

    nc.compile()
    return nc


_cache: dict = {}

# test harnesses may set e.g. {"trace": True, "tmpdir": ...}; empty for grading
_run_opts: dict = {}
LAST_RES = None


def _get_nc(NKV: int):
    if NKV not in _cache:
        _cache[NKV] = _build(NKV)
    return _cache[NKV]


def kernel(query, key_, value, mask, q_w, q_b, k_w, k_b, v_w, v_b, o_w, o_b):
    query = np.asarray(query, np.float32)
    key_ = np.asarray(key_, np.float32)
    value = np.asarray(value, np.float32)
    mask = np.asarray(mask)
    q_w = np.asarray(q_w, np.float32)
    q_b = np.asarray(q_b, np.float32)
    k_w = np.asarray(k_w, np.float32)
    k_b = np.asarray(k_b, np.float32)
    v_w = np.asarray(v_w, np.float32)
    v_b = np.asarray(v_b, np.float32)
    o_w = np.asarray(o_w, np.float32)
    o_b = np.asarray(o_b, np.float32)

    counts = (mask != 0).sum(axis=1)
    NKV = max(P, int(-(-int(counts.max()) // P) * P))
    nc = _get_nc(NKV)

    zeros_ob = np.zeros_like(o_b)
    in_maps = []
    for b in range(BS):
        idx = np.nonzero(mask[b])[0]
        cnt = len(idx)
        xk_g = np.zeros((NKV, DIM), np.float32)
        xv_g = np.zeros((NKV, DIM), np.float32)
        xk_g[:cnt] = key_[b][idx]
        xv_g[:cnt] = value[b][idx]
        xqT_b = np.ascontiguousarray(query[b].T).astype(MM_NP)
        xkT_b = np.ascontiguousarray(xk_g.T).astype(MM_NP)
        xvT_b = np.zeros((P * 7, NKV), MM_NP)
        xvT_b[:DIM] = xv_g.T
        xvT_b[DIM] = 1.0
        pb_b = np.where(np.arange(NKV) < cnt, 0.0, NEG).astype(np.float32)
        for g in range(2):
            sl = slice(DGRP * g, DGRP * (g + 1))
            in_maps.append({
                "xqT": xqT_b,
                "xkT": xkT_b,
                "xvT": xvT_b,
                "wqT": np.ascontiguousarray(q_w[sl].T).astype(MM_NP),
                "wkT": np.ascontiguousarray(k_w[sl].T).astype(MM_NP),
                "wvT": np.concatenate(
                    [v_w[sl].T, v_b[None, sl],
                     np.zeros((P - 1, DGRP), np.float32)], axis=0).astype(MM_NP),
                "woT": np.ascontiguousarray(o_w[:, sl].T).astype(MM_NP),
                "qb": q_b[sl].copy(),
                "kb": k_b[sl].copy(),
                "ob": o_b if g == 0 else zeros_ob,
                "pb": pb_b,
            })

    res = run_bass_kernel_spmd(nc, in_maps, core_ids=list(range(N_CORES)),
                               **_run_opts)
    global LAST_RES
    LAST_RES = res
    out = np.empty((BS, SEQ, DIM), np.float32)
    for b in range(BS):
        out[b] = res.results[2 * b]["out"] + res.results[2 * b + 1]["out"]
    return out



# revision 6
# speedup vs baseline: 1.1306x; 1.1306x over previous
"""Multi-head self-attention (B=4, S=2048, D=768, H=12, dh=64) on 8 trn2 cores.

Sharding: core = b*2 + g  (b = batch 0..3, g = head-group of 6 heads).
Each core computes q/k/v projections for its 6 heads over the full sequence,
masked softmax attention, and a partial output projection (column slice of
o_w => row-parallel). Host sums the two partial outputs per batch element.

v2 design (ACT-exp is the ~115us floor; everything else hides under it):
  - mask gather on host: only unmasked k positions (padded to a multiple of
    128) are shipped/projected/exp'd; pad columns get a -1e30 per-partition
    bias inside the ACT exp instruction (out = exp(scale*s + bias)).
  - scoresT [kv, q] layout; per head the score contraction is K=64 (dh), so
    the two heads of a pair run as CONCURRENT row-tiled matmuls
    (tile_position (0,0) and (64,0)) into separate psum banks - no zero
    padding, up to 2x score throughput.
  - exp on ACT in [128, 1024] tiles; exp tiles persist in SBUF (22-slot
    rotation) so the q-half ctx units can consume them on their own schedule.
  - ctx matmul per head has a 65th 'ones' column in v (from an interleaved
    augmented wv built on host) whose psum row 64 gives the softmax sums for
    free.  ctx runs as two 512-wide halves: half0 per kc inside the unit,
    half1 as a burst after the unit (keeps only one half-unit per psum bank).
  - tails: DVE approx-reciprocal of the sums row, GpSimd partition_broadcast
    of the reciprocal across 64 partitions, one DVE multiply into ctx_sb.
  - psum budget (8 banks): scores A+B = 4, ctx A+B = 2, 2 filler banks for
    q/k/v/out projections paced inside the ACT-bound attention stream.
"""

from collections import deque

import numpy as np
import ml_dtypes

import concourse.bass as bass
import concourse.mybir as mybir
import concourse.tile as tile
from concourse import bacc
from concourse.bass_utils import run_bass_kernel_spmd

BS, SEQ, DIM, NH = 4, 2048, 768, 12
DH = 64
HEADS = 6            # heads per core
NPAIR = 3            # head-pairs per core
DGRP = HEADS * DH    # 384
VGRP = HEADS * 65    # 390 (65-interleaved: 64 ctx cols + ones col per head)
N_CORES = 8
P = 128
QH = 1024            # exp/scores tile width (q)
NT = SEQ // 512      # 4

F32 = mybir.dt.float32
BF16 = mybir.dt.bfloat16

MM_DT = BF16
MM_NP = ml_dtypes.bfloat16 if MM_DT == BF16 else np.float32

NEG = -1.0e30
KIN = DIM // P       # 6 contraction chunks for q/k proj
KIN_V = 7            # 768 inputs + ones row, padded to 896


def _build(NKV: int):
    """Build the per-core Bass program, parameterized by padded kv length."""
    KC = NKV // P          # kv chunks
    QC = SEQ // P          # 16 q chunks for out proj

    nc = bacc.Bacc(None, target_bir_lowering=False, debug=False)

    xqT = nc.declare_dram_parameter("xqT", [DIM, SEQ], MM_DT, isOutput=False)
    xkT = nc.declare_dram_parameter("xkT", [DIM, NKV], MM_DT, isOutput=False)
    xvT = nc.declare_dram_parameter("xvT", [P * KIN_V, NKV], MM_DT, isOutput=False)
    wqT = nc.declare_dram_parameter("wqT", [DIM, DGRP], MM_DT, isOutput=False)
    wkT = nc.declare_dram_parameter("wkT", [DIM, DGRP], MM_DT, isOutput=False)
    wvT = nc.declare_dram_parameter("wvT", [P * KIN_V, VGRP], MM_DT, isOutput=False)
    woT = nc.declare_dram_parameter("woT", [DGRP, DIM], MM_DT, isOutput=False)
    qb = nc.declare_dram_parameter("qb", [DGRP], F32, isOutput=False)
    kb = nc.declare_dram_parameter("kb", [DGRP], F32, isOutput=False)
    ob = nc.declare_dram_parameter("ob", [DIM], F32, isOutput=False)
    pb = nc.declare_dram_parameter("pb", [NKV], F32, isOutput=False)
    out = nc.declare_dram_parameter("out", [SEQ, DIM], F32, isOutput=True)

    xqT_r = xqT.rearrange("(kk pi) n -> pi kk n", pi=P)
    xkT_r = xkT.rearrange("(kk pi) n -> pi kk n", pi=P)
    xvT_r = xvT.rearrange("(kk pi) n -> pi kk n", pi=P)
    wqT_r = wqT.rearrange("(kk pi) n -> pi kk n", pi=P)
    wkT_r = wkT.rearrange("(kk pi) n -> pi kk n", pi=P)
    wvT_r = wvT.rearrange("(kk pi) n -> pi kk n", pi=P)
    woT_r = woT.rearrange("(kk pi) n -> pi kk n", pi=P)
    qb_r = qb.rearrange("(m pi) -> pi m", pi=P)
    kb_r = kb.rearrange("(m pi) -> pi m", pi=P)
    pb_r = pb.rearrange("(c pi) -> pi c", pi=P)

    # k-proj column slices (along kv)
    ksl = []
    o = 0
    while o < NKV:
        w = min(512, NKV - o)
        ksl.append((o, w))
        o += w

    with tile.TileContext(nc) as tc:
        with (
            tc.tile_pool(name="const", bufs=1) as const,
            tc.tile_pool(name="stage", bufs=1) as stage,
            tc.tile_pool(name="persist", bufs=1) as persist,
            tc.tile_pool(name="expp", bufs=1) as expp,
            tc.tile_pool(name="outp", bufs=1) as outp,
            tc.tile_pool(name="stat", bufs=1) as stat,
            tc.tile_pool(name="psS", bufs=1, space="PSUM") as psS,
            tc.tile_pool(name="psC", bufs=1, space="PSUM") as psC,
            tc.tile_pool(name="psF", bufs=1, space="PSUM") as psF,
        ):
            # ---- constants / weights (DMA first; k-proj path is critical) ----
            pb_sb = const.tile([P, KC], F32, name="pb_sb")
            nc.sync.dma_start(pb_sb[:], pb_r)
            qb_sb = const.tile([P, NPAIR], F32, name="qb_sb")
            nc.sync.dma_start(qb_sb[:], qb_r)
            kb_sb = const.tile([P, NPAIR], F32, name="kb_sb")
            nc.sync.dma_start(kb_sb[:], kb_r)
            ob_row = const.tile([1, DIM], F32, name="ob_row")
            nc.sync.dma_start(ob_row[:], ob[None, :])
            wk_sb = const.tile([P, KIN, DGRP], MM_DT, name="wk_sb")
            nc.sync.dma_start(wk_sb[:], wkT_r)
            wq_sb = const.tile([P, KIN, DGRP], MM_DT, name="wq_sb")
            nc.sync.dma_start(wq_sb[:], wqT_r)
            wv_sb = const.tile([P, KIN_V, VGRP], MM_DT, name="wv_sb")
            nc.sync.dma_start(wv_sb[:], wvT_r)
            wo_sb = const.tile([P, NPAIR, DIM], MM_DT, name="wo_sb")
            nc.sync.dma_start(wo_sb[:], woT_r)

            # ---- input staging (per-kk DMAs for queue parallelism) ----
            xk_sb = stage.tile([P, KIN, NKV], MM_DT, name="xk_sb")
            for kk in range(KIN):
                nc.sync.dma_start(xk_sb[:, kk, :], xkT_r[:, kk, :])
            xq_sb = stage.tile([P, KIN, SEQ], MM_DT, name="xq_sb")
            for kk in range(KIN):
                nc.sync.dma_start(xq_sb[:, kk, 0:QH], xqT_r[:, kk, 0:QH])
            xv_sb = stage.tile([P, KIN_V, NKV], MM_DT, name="xv_sb")
            for kk in range(KIN_V):
                nc.sync.dma_start(xv_sb[:, kk, :], xvT_r[:, kk, :])
            for kk in range(KIN):
                nc.sync.dma_start(xq_sb[:, kk, QH:SEQ], xqT_r[:, kk, QH:SEQ])

            # o_b broadcast across partitions (GpSimd; PE-free)
            ob_bc = const.tile([P, DIM], F32, name="ob_bc")
            nc.gpsimd.partition_broadcast(ob_bc[:], ob_row[:])

            # ---- persistent activations ----
            qT_sb = persist.tile([P, NPAIR, SEQ], MM_DT, name="qT_sb")
            kT_sb = persist.tile([P, NPAIR, NKV], MM_DT, name="kT_sb")
            v_sb = persist.tile([P, KC, VGRP], MM_DT, name="v_sb")
            ctx_sb = persist.tile([P, NPAIR, SEQ], MM_DT, name="ctx_sb")

            fctr = [0]

            def vproj_chunk(m):
                fctr[0] += 1
                ps = psF.tile([P, 512], F32, tag=f"f{fctr[0] % 2}",
                              name="psv", bufs=1)
                for kk in range(KIN_V):
                    nc.tensor.matmul(
                        ps[:, 0:VGRP],
                        xv_sb[:, kk, m * P:(m + 1) * P],
                        wv_sb[:, kk, :],
                        start=(kk == 0), stop=(kk == KIN_V - 1),
                    )
                nc.vector.tensor_copy(out=v_sb[:, m, :], in_=ps[:, 0:VGRP])

            def qproj_chunk(p, nt):
                fctr[0] += 1
                ps = psF.tile([P, 512], F32, tag=f"f{fctr[0] % 2}",
                              name="psq", bufs=1)
                for kk in range(KIN):
                    nc.tensor.matmul(
                        ps[:],
                        wq_sb[:, kk, p * P:(p + 1) * P],
                        xq_sb[:, kk, nt * 512:(nt + 1) * 512],
                        start=(kk == 0), stop=(kk == KIN - 1),
                    )
                nc.vector.tensor_scalar_add(
                    qT_sb[:, p, nt * 512:(nt + 1) * 512], ps[:],
                    qb_sb[:, p, None],
                )

            def kproj_chunk(p, si):
                fctr[0] += 1
                o0, w = ksl[si]
                ps = psF.tile([P, 512], F32, tag=f"f{fctr[0] % 2}",
                              name="psk", bufs=1)
                for kk in range(KIN):
                    nc.tensor.matmul(
                        ps[:, 0:w],
                        wk_sb[:, kk, p * P:(p + 1) * P],
                        xk_sb[:, kk, o0:o0 + w],
                        start=(kk == 0), stop=(kk == KIN - 1),
                    )
                nc.vector.tensor_scalar_add(
                    kT_sb[:, p, o0:o0 + w], ps[:, 0:w],
                    kb_sb[:, p, None],
                )

            def outproj_chunk(qc):
                o_t = outp.tile([P, DIM], F32, tag="o", name="o_t", bufs=3)
                for n0 in (0, 384):
                    fctr[0] += 1
                    ps = psF.tile([P, 512], F32, tag=f"f{fctr[0] % 2}",
                                  name="pso", bufs=1)
                    for kk in range(NPAIR):
                        nc.tensor.matmul(
                            ps[:, 0:384],
                            ctx_sb[:, kk, qc * P:(qc + 1) * P],
                            wo_sb[:, kk, n0:n0 + 384],
                            start=(kk == 0), stop=(kk == NPAIR - 1),
                        )
                    nc.vector.tensor_tensor(
                        o_t[:, n0:n0 + 384], ps[:, 0:384],
                        ob_bc[:, n0:n0 + 384],
                        mybir.AluOpType.add,
                    )
                nc.sync.dma_start(out[qc * P:(qc + 1) * P, :], o_t[:])

            # ---- prefix: k/q proj for pairs 0,1 and v chunks 0-2 ----
            for si in range(len(ksl)):
                kproj_chunk(0, si)
            qproj_chunk(0, 0)
            qproj_chunk(0, 1)
            for si in range(len(ksl)):
                kproj_chunk(1, si)
            qproj_chunk(1, 0)
            qproj_chunk(1, 1)
            for m in range(min(3, KC)):
                vproj_chunk(m)

            # ---- filler queues, paced inside the attention stream ----
            Qv = deque(range(3, KC))
            Qproj = deque(
                [(2, lambda si=si: kproj_chunk(2, si)) for si in range(len(ksl))]
                + [(2, lambda: qproj_chunk(2, 0)), (2, lambda: qproj_chunk(2, 1))]
                + [(3, lambda: qproj_chunk(0, 2)), (3, lambda: qproj_chunk(0, 3)),
                   (4, lambda: qproj_chunk(1, 2)), (4, lambda: qproj_chunk(1, 3)),
                   (5, lambda: qproj_chunk(2, 2)), (5, lambda: qproj_chunk(2, 3))]
            )
            Qout = deque()

            def pace_fillers(u, kc):
                # keep v-proj 3 chunks ahead of ctx consumption
                while Qv and Qv[0] <= kc + 3:
                    vproj_chunk(Qv.popleft())
                if Qproj and kc % 2 == 0:
                    Qproj.popleft()[1]()
                elif Qout and kc % 2 == 1:
                    outproj_chunk(Qout.popleft())

            def flush_due(u):
                while Qv and u >= 1:
                    vproj_chunk(Qv.popleft())
                while Qproj and Qproj[0][0] <= u:
                    Qproj.popleft()[1]()

            # ---- attention ----
            units = [(qh, p) for qh in range(SEQ // QH) for p in range(NPAIR)]
            exp_tiles = {}

            def tail(p, qh, half, h, ps_ctx):
                q0 = qh * QH + half * 512
                sums_t = stat.tile([1, 512], F32, tag="s", name="sums_t",
                                   bufs=4)
                nc.vector.tensor_copy(out=sums_t[:], in_=ps_ctx[64:65, :])
                recip_t = stat.tile([1, 512], F32, tag="r", name="recip_t",
                                    bufs=4)
                nc.vector.reciprocal_approx_fast(
                    out=recip_t[:], in_=sums_t[:])
                rbc = stat.tile([DH, 512], F32, tag="rb", name="rbc", bufs=4)
                nc.gpsimd.partition_broadcast(rbc[:], recip_t[:])
                nc.vector.tensor_tensor(
                    ctx_sb[64 * h:64 * h + DH, p, q0:q0 + 512],
                    ps_ctx[0:DH, :],
                    rbc[:],
                    mybir.AluOpType.mult,
                )

            ctx_live = {}

            def ctx_half(u, half, kcs):
                qh, p = units[u]
                key = (u, half)
                if key not in ctx_live:
                    ctx_live[key] = (
                        psC.tile([P, 512], F32, tag="cA", name="pscA", bufs=1),
                        psC.tile([P, 512], F32, tag="cB", name="pscB", bufs=1),
                    )
                tiles = ctx_live[key]
                for kc in kcs:
                    for h in range(2):
                        g = 2 * p + h
                        nc.tensor.matmul(
                            tiles[h][0:65, :],
                            v_sb[:, kc, 65 * g:65 * g + 65],
                            exp_tiles[(u, h, kc)][:, half * 512:(half + 1) * 512],
                            start=(kc == 0), stop=(kc == KC - 1),
                        )
                if kcs[-1] == KC - 1:
                    for h in range(2):
                        tail(p, qh, half, h, tiles[h])
                    del ctx_live[key]

            def make_burst(u):
                qh, p = units[u]

                def burst():
                    ctx_half(u, 1, list(range(KC)))
                    for h in range(2):
                        for kc in range(KC):
                            exp_tiles.pop((u, h, kc), None)
                    if p == NPAIR - 1:
                        Qout.extend(range(qh * (QC // 2), (qh + 1) * (QC // 2)))
                return burst

            pending_burst = None
            for u, (qh, p) in enumerate(units):
                flush_due(u)
                for kc in range(KC):
                    ps_s = [psS.tile([P, QH], F32, tag="sA", name="pssA", bufs=1),
                            psS.tile([P, QH], F32, tag="sB", name="pssB", bufs=1)]
                    for qt in range(QH // 512):
                        c0 = qh * QH + qt * 512
                        for h in range(2):
                            nc.tensor.matmul(
                                ps_s[h][:, qt * 512:(qt + 1) * 512],
                                kT_sb[64 * h:64 * (h + 1), p,
                                      kc * P:(kc + 1) * P],
                                qT_sb[64 * h:64 * (h + 1), p, c0:c0 + 512],
                                start=True, stop=True,
                                tile_position=(64 * h, 0),
                            )
                    for h in range(2):
                        e_t = expp.tile([P, QH], MM_DT, tag="e", name="e_t",
                                        bufs=22)
                        nc.scalar.activation(
                            e_t[:], ps_s[h][:],
                            mybir.ActivationFunctionType.Exp,
                            bias=pb_sb[:, kc, None], scale=0.125,
                        )
                        exp_tiles[(u, h, kc)] = e_t
                    # previous unit's half1 burst goes right after this unit's
                    # first scores+exp, BEFORE ctx_half(u,0) allocates its psC
                    # banks (rotation order: ...(u-1,0),(u-1,1),(u,0)...).
                    if kc == 0 and pending_burst is not None:
                        pending_burst()
                        pending_burst = None
                    ctx_half(u, 0, [kc])
                    pace_fillers(u, kc)
                pending_burst = make_burst(u)
            pending_burst()

            # ---- suffix: flush remaining fillers and out-proj ----
            while Qv:
                vproj_chunk(Qv.popleft())
            while Qproj:
                Qproj.popleft()[1]()
            while Qout:
                outproj_chunk(Qout.popleft())

    nc.compile()
    return nc


_cache: dict = {}

# test harnesses may set e.g. {"trace": True, "tmpdir": ...}; empty for grading
_run_opts: dict = {}
LAST_RES = None


def _get_nc(NKV: int):
    if NKV not in _cache:
        _cache[NKV] = _build(NKV)
    return _cache[NKV]


def kernel(query, key_, value, mask, q_w, q_b, k_w, k_b, v_w, v_b, o_w, o_b):
    query = np.asarray(query, np.float32)
    key_ = np.asarray(key_, np.float32)
    value = np.asarray(value, np.float32)
    mask = np.asarray(mask)
    q_w = np.asarray(q_w, np.float32)
    q_b = np.asarray(q_b, np.float32)
    k_w = np.asarray(k_w, np.float32)
    k_b = np.asarray(k_b, np.float32)
    v_w = np.asarray(v_w, np.float32)
    v_b = np.asarray(v_b, np.float32)
    o_w = np.asarray(o_w, np.float32)
    o_b = np.asarray(o_b, np.float32)

    counts = (mask != 0).sum(axis=1)
    NKV = max(P, int(-(-int(counts.max()) // P) * P))
    nc = _get_nc(NKV)

    zeros_ob = np.zeros_like(o_b)
    in_maps = []
    for b in range(BS):
        idx = np.nonzero(mask[b])[0]
        cnt = len(idx)
        xk_g = np.zeros((NKV, DIM), np.float32)
        xv_g = np.zeros((NKV, DIM), np.float32)
        xk_g[:cnt] = key_[b][idx]
        xv_g[:cnt] = value[b][idx]
        xqT_b = np.ascontiguousarray(query[b].T).astype(MM_NP)
        xkT_b = np.ascontiguousarray(xk_g.T).astype(MM_NP)
        xvT_b = np.zeros((P * KIN_V, NKV), MM_NP)
        xvT_b[:DIM] = xv_g.T
        xvT_b[DIM] = 1.0
        pb_b = np.where(np.arange(NKV) < cnt, 0.0, NEG).astype(np.float32)
        for g in range(2):
            sl = slice(DGRP * g, DGRP * (g + 1))
            # interleaved augmented wv: col 65h+j (j<64) = v_w.T col, rows
            # 0-767; row 768 = v_b;  col 65h+64 = ones-selector (row 768 = 1).
            wv_aug = np.zeros((P * KIN_V, VGRP), np.float32)
            vwT = v_w[sl].T  # [768, 384]
            vb = v_b[sl]
            for h in range(HEADS):
                wv_aug[:DIM, 65 * h:65 * h + 64] = vwT[:, 64 * h:64 * h + 64]
                wv_aug[DIM, 65 * h:65 * h + 64] = vb[64 * h:64 * h + 64]
                wv_aug[DIM, 65 * h + 64] = 1.0
            in_maps.append({
                "xqT": xqT_b,
                "xkT": xkT_b,
                "xvT": xvT_b,
                "wqT": np.ascontiguousarray(q_w[sl].T).astype(MM_NP),
                "wkT": np.ascontiguousarray(k_w[sl].T).astype(MM_NP),
                "wvT": wv_aug.astype(MM_NP),
                "woT": np.ascontiguousarray(o_w[:, sl].T).astype(MM_NP),
                "qb": q_b[sl].copy(),
                "kb": k_b[sl].copy(),
                "ob": o_b if g == 0 else zeros_ob,
                "pb": pb_b,
            })

    res = run_bass_kernel_spmd(nc, in_maps, core_ids=list(range(N_CORES)),
                               **_run_opts)
    global LAST_RES
    LAST_RES = res
    out = np.empty((BS, SEQ, DIM), np.float32)
    for b in range(BS):
        out[b] = res.results[2 * b]["out"] + res.results[2 * b + 1]["out"]
    return out


# revision 9
# speedup vs baseline: 1.1614x; 1.0273x over previous
"""Multi-head self-attention (B=4, S=2048, D=768, H=12, dh=64) on 8 trn2 cores.

Sharding: core = b*2 + g  (b = batch 0..3, g = head-group of 6 heads).
Each core computes q/k/v projections for its 6 heads over the full sequence,
masked softmax attention, and a partial output projection (column slice of
o_w => row-parallel). Host sums the two partial outputs per batch element.

v2 design (ACT-exp is the ~115us floor; everything else hides under it):
  - mask gather on host: only unmasked k positions (padded to a multiple of
    128) are shipped/projected/exp'd; pad columns get a -1e30 per-partition
    bias inside the ACT exp instruction (out = exp(scale*s + bias)).
  - scoresT [kv, q] layout; per head the score contraction is K=64 (dh), so
    the two heads of a pair run as CONCURRENT row-tiled matmuls
    (tile_position (0,0) and (64,0)) into separate psum banks - no zero
    padding, up to 2x score throughput.
  - exp on ACT in [128, 1024] tiles; exp tiles persist in SBUF (22-slot
    rotation) so the q-half ctx units can consume them on their own schedule.
  - ctx matmul per head has a 65th 'ones' column in v (from an interleaved
    augmented wv built on host) whose psum row 64 gives the softmax sums for
    free.  ctx runs as two 512-wide halves: half0 per kc inside the unit,
    half1 as a burst after the unit (keeps only one half-unit per psum bank).
  - tails: DVE approx-reciprocal of the sums row, GpSimd partition_broadcast
    of the reciprocal across 64 partitions, one DVE multiply into ctx_sb.
  - psum budget (8 banks): scores A+B = 4, ctx A+B = 2, 2 filler banks for
    q/k/v/out projections paced inside the ACT-bound attention stream.
"""

from collections import deque

import numpy as np
import ml_dtypes

import concourse.bass as bass
import concourse.mybir as mybir
import concourse.tile as tile
from concourse import bacc
from concourse.bass_utils import run_bass_kernel_spmd

BS, SEQ, DIM, NH = 4, 2048, 768, 12
DH = 64
HEADS = 6            # heads per core
NPAIR = 3            # head-pairs per core
DGRP = HEADS * DH    # 384
VGRP = HEADS * 65    # 390 (65-interleaved: 64 ctx cols + ones col per head)
N_CORES = 8
P = 128
QH = 1024            # exp/scores tile width (q)
NT = SEQ // 512      # 4

F32 = mybir.dt.float32
BF16 = mybir.dt.bfloat16

MM_DT = BF16
MM_NP = ml_dtypes.bfloat16 if MM_DT == BF16 else np.float32

NEG = -1.0e30
KIN = DIM // P       # 6 contraction chunks for q/k proj
KIN_V = 7            # 768 inputs + ones row, padded to 896


def _build(NKV: int):
    """Build the per-core Bass program, parameterized by padded kv length."""
    KC = NKV // P          # kv chunks
    QC = SEQ // P          # 16 q chunks for out proj

    nc = bacc.Bacc(None, target_bir_lowering=False, debug=False)

    xqT = nc.declare_dram_parameter("xqT", [DIM, SEQ], MM_DT, isOutput=False)
    xkT = nc.declare_dram_parameter("xkT", [DIM, NKV], MM_DT, isOutput=False)
    xvT = nc.declare_dram_parameter("xvT", [P * KIN_V, NKV], MM_DT, isOutput=False)
    wqT = nc.declare_dram_parameter("wqT", [DIM, DGRP], MM_DT, isOutput=False)
    wkT = nc.declare_dram_parameter("wkT", [DIM, DGRP], MM_DT, isOutput=False)
    wvT = nc.declare_dram_parameter("wvT", [P * KIN_V, VGRP], MM_DT, isOutput=False)
    woT = nc.declare_dram_parameter("woT", [DGRP, DIM], MM_DT, isOutput=False)
    qb = nc.declare_dram_parameter("qb", [DGRP], F32, isOutput=False)
    kb = nc.declare_dram_parameter("kb", [DGRP], F32, isOutput=False)
    ob = nc.declare_dram_parameter("ob", [DIM], F32, isOutput=False)
    pb = nc.declare_dram_parameter("pb", [NKV], F32, isOutput=False)
    out = nc.declare_dram_parameter("out", [SEQ, DIM], F32, isOutput=True)

    xqT_r = xqT.rearrange("(kk pi) n -> pi kk n", pi=P)
    xkT_r = xkT.rearrange("(kk pi) n -> pi kk n", pi=P)
    xvT_r = xvT.rearrange("(kk pi) n -> pi kk n", pi=P)
    wqT_r = wqT.rearrange("(kk pi) n -> pi kk n", pi=P)
    wkT_r = wkT.rearrange("(kk pi) n -> pi kk n", pi=P)
    wvT_r = wvT.rearrange("(kk pi) n -> pi kk n", pi=P)
    woT_r = woT.rearrange("(kk pi) n -> pi kk n", pi=P)
    qb_r = qb.rearrange("(m pi) -> pi m", pi=P)
    kb_r = kb.rearrange("(m pi) -> pi m", pi=P)
    pb_r = pb.rearrange("(c pi) -> pi c", pi=P)

    # k-proj column slices (along kv)
    ksl = []
    o = 0
    while o < NKV:
        w = min(512, NKV - o)
        ksl.append((o, w))
        o += w

    with tile.TileContext(nc) as tc:
        with (
            tc.tile_pool(name="const", bufs=1) as const,
            tc.tile_pool(name="stage", bufs=1) as stage,
            tc.tile_pool(name="persist", bufs=1) as persist,
            tc.tile_pool(name="expp", bufs=1) as expp,
            tc.tile_pool(name="outp", bufs=1) as outp,
            tc.tile_pool(name="stat", bufs=1) as stat,
            tc.tile_pool(name="psS", bufs=1, space="PSUM") as psS,
            tc.tile_pool(name="psC", bufs=1, space="PSUM") as psC,
            tc.tile_pool(name="psF", bufs=1, space="PSUM") as psF,
        ):
            # ---- constants / weights (DMA first; k-proj path is critical) ----
            pb_sb = const.tile([P, KC], F32, name="pb_sb")
            nc.sync.dma_start(pb_sb[:], pb_r)
            qb_sb = const.tile([P, NPAIR], F32, name="qb_sb")
            nc.sync.dma_start(qb_sb[:], qb_r)
            kb_sb = const.tile([P, NPAIR], F32, name="kb_sb")
            nc.sync.dma_start(kb_sb[:], kb_r)
            ob_row = const.tile([1, DIM], F32, name="ob_row")
            nc.sync.dma_start(ob_row[:], ob[None, :])
            wk_sb = const.tile([P, KIN, DGRP], MM_DT, name="wk_sb")
            nc.sync.dma_start(wk_sb[:], wkT_r)
            wq_sb = const.tile([P, KIN, DGRP], MM_DT, name="wq_sb")
            nc.sync.dma_start(wq_sb[:], wqT_r)
            wv_sb = const.tile([P, KIN_V, VGRP], MM_DT, name="wv_sb")
            nc.sync.dma_start(wv_sb[:], wvT_r)
            wo_sb = const.tile([P, NPAIR, DIM], MM_DT, name="wo_sb")
            nc.sync.dma_start(wo_sb[:], woT_r)

            # ---- input staging (per-kk DMAs for queue parallelism) ----
            xk_sb = stage.tile([P, KIN, NKV], MM_DT, name="xk_sb")
            for kk in range(KIN):
                nc.sync.dma_start(xk_sb[:, kk, :], xkT_r[:, kk, :])
            xq_sb = stage.tile([P, KIN, SEQ], MM_DT, name="xq_sb")
            for kk in range(KIN):
                nc.sync.dma_start(xq_sb[:, kk, 0:QH], xqT_r[:, kk, 0:QH])
            xv_sb = stage.tile([P, KIN_V, NKV], MM_DT, name="xv_sb")
            for kk in range(KIN_V):
                nc.sync.dma_start(xv_sb[:, kk, :], xvT_r[:, kk, :])
            for kk in range(KIN):
                nc.sync.dma_start(xq_sb[:, kk, QH:SEQ], xqT_r[:, kk, QH:SEQ])

            # o_b broadcast across partitions (GpSimd; PE-free)
            ob_bc = const.tile([P, DIM], F32, name="ob_bc")
            nc.gpsimd.partition_broadcast(ob_bc[:], ob_row[:])

            # ---- persistent activations ----
            qT_sb = persist.tile([P, NPAIR, SEQ], MM_DT, name="qT_sb")
            kT_sb = persist.tile([P, NPAIR, NKV], MM_DT, name="kT_sb")
            v_sb = persist.tile([P, KC, VGRP], MM_DT, name="v_sb")
            ctx_sb = persist.tile([P, NPAIR, SEQ], MM_DT, name="ctx_sb")

            fctr = [0]

            def vproj_chunk(m):
                fctr[0] += 1
                ps = psF.tile([P, 512], F32, tag=f"f{fctr[0] % 2}",
                              name="psv", bufs=1)
                for kk in range(KIN_V):
                    nc.tensor.matmul(
                        ps[:, 0:VGRP],
                        xv_sb[:, kk, m * P:(m + 1) * P],
                        wv_sb[:, kk, :],
                        start=(kk == 0), stop=(kk == KIN_V - 1),
                    )
                nc.vector.tensor_copy(out=v_sb[:, m, :], in_=ps[:, 0:VGRP])

            def qproj_chunk(p, nt):
                fctr[0] += 1
                ps = psF.tile([P, 512], F32, tag=f"f{fctr[0] % 2}",
                              name="psq", bufs=1)
                for kk in range(KIN):
                    nc.tensor.matmul(
                        ps[:],
                        wq_sb[:, kk, p * P:(p + 1) * P],
                        xq_sb[:, kk, nt * 512:(nt + 1) * 512],
                        start=(kk == 0), stop=(kk == KIN - 1),
                    )
                nc.vector.tensor_scalar_add(
                    qT_sb[:, p, nt * 512:(nt + 1) * 512], ps[:],
                    qb_sb[:, p, None],
                )

            def kproj_chunk(p, si):
                fctr[0] += 1
                o0, w = ksl[si]
                ps = psF.tile([P, 512], F32, tag=f"f{fctr[0] % 2}",
                              name="psk", bufs=1)
                for kk in range(KIN):
                    nc.tensor.matmul(
                        ps[:, 0:w],
                        wk_sb[:, kk, p * P:(p + 1) * P],
                        xk_sb[:, kk, o0:o0 + w],
                        start=(kk == 0), stop=(kk == KIN - 1),
                    )
                nc.vector.tensor_scalar_add(
                    kT_sb[:, p, o0:o0 + w], ps[:, 0:w],
                    kb_sb[:, p, None],
                )

            def outproj_chunk(qc):
                o_t = outp.tile([P, DIM], F32, tag="o", name="o_t", bufs=3)
                for n0 in (0, 384):
                    fctr[0] += 1
                    ps = psF.tile([P, 512], F32, tag=f"f{fctr[0] % 2}",
                                  name="pso", bufs=1)
                    for kk in range(NPAIR):
                        nc.tensor.matmul(
                            ps[:, 0:384],
                            ctx_sb[:, kk, qc * P:(qc + 1) * P],
                            wo_sb[:, kk, n0:n0 + 384],
                            start=(kk == 0), stop=(kk == NPAIR - 1),
                        )
                    nc.vector.tensor_tensor(
                        o_t[:, n0:n0 + 384], ps[:, 0:384],
                        ob_bc[:, n0:n0 + 384],
                        mybir.AluOpType.add,
                    )
                nc.sync.dma_start(out[qc * P:(qc + 1) * P, :], o_t[:])

            # ---- prefix: minimal critical path for unit 0 ----
            for si in range(len(ksl)):
                kproj_chunk(0, si)
            qproj_chunk(0, 0)
            qproj_chunk(0, 1)
            vproj_chunk(0)
            vproj_chunk(1)

            # ---- filler queues, paced inside the attention stream ----
            Qv = deque(range(2, KC))
            Qproj = deque(
                [(1, lambda si=si: kproj_chunk(1, si)) for si in range(len(ksl))]
                + [(1, lambda: qproj_chunk(1, 0)), (1, lambda: qproj_chunk(1, 1))]
                + [(2, lambda si=si: kproj_chunk(2, si)) for si in range(len(ksl))]
                + [(2, lambda: qproj_chunk(2, 0)), (2, lambda: qproj_chunk(2, 1))]
                + [(3, lambda: qproj_chunk(0, 2)), (3, lambda: qproj_chunk(0, 3)),
                   (4, lambda: qproj_chunk(1, 2)), (4, lambda: qproj_chunk(1, 3)),
                   (5, lambda: qproj_chunk(2, 2)), (5, lambda: qproj_chunk(2, 3))]
            )
            Qout = deque()

            def pace_fillers(u, kc):
                # keep v-proj 4 chunks ahead of (lagged) ctx consumption
                while Qv and Qv[0] <= kc + 2:
                    vproj_chunk(Qv.popleft())
                if Qproj:
                    Qproj.popleft()[1]()
                elif Qout and kc % 2 == 1:
                    outproj_chunk(Qout.popleft())

            def flush_due(u):
                while Qproj and Qproj[0][0] <= u:
                    Qproj.popleft()[1]()

            # ---- attention ----
            units = [(qh, p) for qh in range(SEQ // QH) for p in range(NPAIR)]
            exp_tiles = {}
            LAG = 2

            def tail(p, qh, half, h, ps_ctx):
                q0 = qh * QH + half * 512
                sums_t = stat.tile([1, 512], F32, tag="s", name="sums_t",
                                   bufs=4)
                nc.vector.tensor_copy(out=sums_t[:], in_=ps_ctx[64:65, :])
                recip_t = stat.tile([1, 512], F32, tag="r", name="recip_t",
                                    bufs=4)
                nc.vector.reciprocal_approx_fast(
                    out=recip_t[:], in_=sums_t[:])
                rbc = stat.tile([DH, 512], F32, tag="rb", name="rbc", bufs=4)
                nc.gpsimd.partition_broadcast(rbc[:], recip_t[:])
                nc.vector.tensor_tensor(
                    ctx_sb[64 * h:64 * h + DH, p, q0:q0 + 512],
                    ps_ctx[0:DH, :],
                    rbc[:],
                    mybir.AluOpType.mult,
                )

            ctx_live = {}

            def ctx_item(u, half, kc):
                qh, p = units[u]
                key = (u, half)
                if key not in ctx_live:
                    ctx_live[key] = (
                        psC.tile([P, 512], F32, tag="cA", name="pscA", bufs=1),
                        psC.tile([P, 512], F32, tag="cB", name="pscB", bufs=1),
                    )
                tiles = ctx_live[key]
                for h in range(2):
                    g = 2 * p + h
                    nc.tensor.matmul(
                        tiles[h][0:65, :],
                        v_sb[:, kc, 65 * g:65 * g + 65],
                        exp_tiles[(u, h, kc)][:, half * 512:(half + 1) * 512],
                        start=(kc == 0), stop=(kc == KC - 1),
                    )
                    if half == 1:
                        exp_tiles.pop((u, h, kc), None)
                if kc == KC - 1:
                    for h in range(2):
                        tail(p, qh, half, h, tiles[h])
                    del ctx_live[key]

            ctxq = deque()

            def pop_ctx(nmax, keep=LAG):
                n = 0
                while ctxq and len(ctxq) > keep and n < nmax:
                    ctx_item(*ctxq.popleft())
                    n += 1

            for u, (qh, p) in enumerate(units):
                flush_due(u)
                for kc in range(KC):
                    ps_s = [psS.tile([P, QH], F32, tag="sA", name="pssA", bufs=1),
                            psS.tile([P, QH], F32, tag="sB", name="pssB", bufs=1)]
                    for qt in range(QH // 512):
                        c0 = qh * QH + qt * 512
                        for h in range(2):
                            nc.tensor.matmul(
                                ps_s[h][:, qt * 512:(qt + 1) * 512],
                                kT_sb[64 * h:64 * (h + 1), p,
                                      kc * P:(kc + 1) * P],
                                qT_sb[64 * h:64 * (h + 1), p, c0:c0 + 512],
                                start=True, stop=True,
                                tile_position=(64 * h, 0),
                            )
                    for h in range(2):
                        e_t = expp.tile([P, QH], MM_DT, tag="e", name="e_t",
                                        bufs=22)
                        nc.scalar.activation(
                            e_t[:], ps_s[h][:],
                            mybir.ActivationFunctionType.Exp,
                            bias=pb_sb[:, kc, None], scale=0.125,
                        )
                        exp_tiles[(u, h, kc)] = e_t
                    ctxq.append((u, 0, kc))
                    pop_ctx(3)
                    pace_fillers(u, kc)
                for kc in range(KC):
                    ctxq.append((u, 1, kc))
                if p == NPAIR - 1:
                    Qout.extend(range(qh * (QC // 2), (qh + 1) * (QC // 2)))
            while ctxq:
                ctx_item(*ctxq.popleft())

            # ---- suffix: flush remaining fillers and out-proj ----
            while Qv:
                vproj_chunk(Qv.popleft())
            while Qproj:
                Qproj.popleft()[1]()
            while Qout:
                outproj_chunk(Qout.popleft())

    nc.compile()
    return nc


_cache: dict = {}

# test harnesses may set e.g. {"trace": True, "tmpdir": ...}; empty for grading
_run_opts: dict = {}
LAST_RES = None


def _get_nc(NKV: int):
    if NKV not in _cache:
        _cache[NKV] = _build(NKV)
    return _cache[NKV]


def kernel(query, key_, value, mask, q_w, q_b, k_w, k_b, v_w, v_b, o_w, o_b):
    query = np.asarray(query, np.float32)
    key_ = np.asarray(key_, np.float32)
    value = np.asarray(value, np.float32)
    mask = np.asarray(mask)
    q_w = np.asarray(q_w, np.float32)
    q_b = np.asarray(q_b, np.float32)
    k_w = np.asarray(k_w, np.float32)
    k_b = np.asarray(k_b, np.float32)
    v_w = np.asarray(v_w, np.float32)
    v_b = np.asarray(v_b, np.float32)
    o_w = np.asarray(o_w, np.float32)
    o_b = np.asarray(o_b, np.float32)

    counts = (mask != 0).sum(axis=1)
    NKV = max(P, int(-(-int(counts.max()) // P) * P))
    nc = _get_nc(NKV)

    zeros_ob = np.zeros_like(o_b)
    in_maps = []
    for b in range(BS):
        idx = np.nonzero(mask[b])[0]
        cnt = len(idx)
        xk_g = np.zeros((NKV, DIM), np.float32)
        xv_g = np.zeros((NKV, DIM), np.float32)
        xk_g[:cnt] = key_[b][idx]
        xv_g[:cnt] = value[b][idx]
        xqT_b = np.ascontiguousarray(query[b].T).astype(MM_NP)
        xkT_b = np.ascontiguousarray(xk_g.T).astype(MM_NP)
        xvT_b = np.zeros((P * KIN_V, NKV), MM_NP)
        xvT_b[:DIM] = xv_g.T
        xvT_b[DIM] = 1.0
        pb_b = np.where(np.arange(NKV) < cnt, 0.0, NEG).astype(np.float32)
        for g in range(2):
            sl = slice(DGRP * g, DGRP * (g + 1))
            # interleaved augmented wv: col 65h+j (j<64) = v_w.T col, rows
            # 0-767; row 768 = v_b;  col 65h+64 = ones-selector (row 768 = 1).
            wv_aug = np.zeros((P * KIN_V, VGRP), np.float32)
            vwT = v_w[sl].T  # [768, 384]
            vb = v_b[sl]
            for h in range(HEADS):
                wv_aug[:DIM, 65 * h:65 * h + 64] = vwT[:, 64 * h:64 * h + 64]
                wv_aug[DIM, 65 * h:65 * h + 64] = vb[64 * h:64 * h + 64]
                wv_aug[DIM, 65 * h + 64] = 1.0
            in_maps.append({
                "xqT": xqT_b,
                "xkT": xkT_b,
                "xvT": xvT_b,
                "wqT": np.ascontiguousarray(q_w[sl].T).astype(MM_NP),
                "wkT": np.ascontiguousarray(k_w[sl].T).astype(MM_NP),
                "wvT": wv_aug.astype(MM_NP),
                "woT": np.ascontiguousarray(o_w[:, sl].T).astype(MM_NP),
                "qb": q_b[sl].copy(),
                "kb": k_b[sl].copy(),
                "ob": o_b if g == 0 else zeros_ob,
                "pb": pb_b,
            })

    res = run_bass_kernel_spmd(nc, in_maps, core_ids=list(range(N_CORES)),
                               **_run_opts)
    global LAST_RES
    LAST_RES = res
    out = np.empty((BS, SEQ, DIM), np.float32)
    for b in range(BS):
        out[b] = res.results[2 * b]["out"] + res.results[2 * b + 1]["out"]
    return out
